# revision 1
# baseline (speedup 1.0000x reference)
"""Trainium2 Bass kernel for multi-head attention (B=2, L=2048, D=1024, H=16).

Sharding: 8 cores = 2 (batch) x 4 (head-groups of 4 heads).  Each core
computes q/k/v projections for its 4 heads, softmax attention, and a
partial output projection against its 256 columns of W_o.  The all-reduce
of the 4 partials per batch happens on the host (free).

All matmuls run in bf16 with fp32 PSUM accumulation.  Softmax skips the
max-subtraction (scores are ~N(0, 1/3); exp is safely in range).
"""

import sys

if "/opt/trn_rl_repo" not in sys.path:
    sys.path.insert(0, "/opt/trn_rl_repo")

import numpy as np
import ml_dtypes

import concourse.bass as bass
import concourse.mybir as mybir
import concourse.tile as tile
from concourse import bacc
from concourse.bass_utils import run_bass_kernel_spmd

B, L, D, H = 2, 2048, 1024, 16
HD = D // H          # 64 head dim
NH = 4               # heads per core
GW = NH * HD         # 256 group width
SCALE = (H / D) ** 0.5  # 1/8
P = 128
KT = D // P          # 8 contraction tiles over D
TBLK = L // P        # 16 token blocks of 128
QC = L // 512        # 4 query chunks of 512
BF16 = mybir.dt.bfloat16
F32 = mybir.dt.float32
EXP = mybir.ActivationFunctionType.Exp

PEXP_BUFS = 25       # P' slots: see v9 slot math (full chains at k=0,3,6,9)


def _build():
    nc = bacc.Bacc(None, target_bir_lowering=False, debug=False)

    xT_d = nc.dram_tensor("xT", (D, L), BF16, kind="ExternalInput")
    wqT_d = nc.dram_tensor("wqT", (D, GW), BF16, kind="ExternalInput")
    wkT_d = nc.dram_tensor("wkT", (D, GW), BF16, kind="ExternalInput")
    wvT_d = nc.dram_tensor("wvT", (D, GW), BF16, kind="ExternalInput")
    woT_d = nc.dram_tensor("woT", (GW, D), BF16, kind="ExternalInput")
    out_d = nc.dram_tensor("out", (L, D), BF16, kind="ExternalOutput")

    with tile.TileContext(nc) as tc:
        with (
            tc.tile_pool(name="persist", bufs=1) as pers,
            tc.tile_pool(name="pexp", bufs=PEXP_BUFS) as pexp,
            tc.tile_pool(name="oeT", bufs=2) as oep,
            tc.tile_pool(name="rcp", bufs=4) as rcpp,
            tc.tile_pool(name="srow", bufs=4) as srp,
            tc.tile_pool(name="osb", bufs=2) as osbp,
            tc.tile_pool(name="spsum", bufs=3, space="PSUM") as sps,
            tc.tile_pool(name="accp", bufs=2, space="PSUM") as accp,
        ):
            # ---- persistent SBUF tensors ----
            xT = [pers.tile([P, L], BF16, tag=f"xT{k}", name=f"xT{k}") for k in range(KT)]
            wqT = [pers.tile([P, GW], BF16, tag=f"wqT{k}", name=f"wqT{k}") for k in range(KT)]
            wkT = [pers.tile([P, GW], BF16, tag=f"wkT{k}", name=f"wkT{k}") for k in range(KT)]
            wvT = [pers.tile([P, GW], BF16, tag=f"wvT{k}", name=f"wvT{k}") for k in range(KT)]
            woT = [pers.tile([P, D], BF16, tag=f"woT{i}", name=f"woT{i}") for i in range(GW // P)]
            qT = [pers.tile([P, L], BF16, tag=f"qT{m}", name=f"qT{m}") for m in range(GW // P)]
            kTt = [pers.tile([P, L], BF16, tag=f"kT{m}", name=f"kT{m}") for m in range(GW // P)]
            vext = [pers.tile([P, NH * (HD + 1)], BF16, tag=f"vx{t}", name=f"vx{t}") for t in range(TBLK)]
            aoT = [pers.tile([P, L], BF16, tag=f"aoT{m}", name=f"aoT{m}") for m in range(GW // P)]
            ones64 = pers.tile([1, HD], BF16, tag="ones64")
            nc.any.memset(ones64[:], 1.0)
            warm = pers.tile([1, 2], BF16, tag="warm")
            nc.scalar.activation(warm[:], ones64[:, 0:2], EXP)  # preload exp table

            for k in range(KT):
                nc.sync.dma_start(xT[k][:], xT_d[k * P:(k + 1) * P, :])
                nc.sync.dma_start(wqT[k][:], wqT_d[k * P:(k + 1) * P, :])
                nc.sync.dma_start(wkT[k][:], wkT_d[k * P:(k + 1) * P, :])
            for k in range(KT):
                nc.sync.dma_start(wvT[k][:], wvT_d[k * P:(k + 1) * P, :])
            for i in range(GW // P):
                nc.sync.dma_start(woT[i][:], woT_d[i * P:(i + 1) * P, :])

            # ---- helper emitters ----
            def emit_proj_chain(dst, w, m, tck):
                """dst[m][:, tck*512:+512] = (W[m-block] @ x^T)[:, chunk], accum over K."""
                ps = accp.tile([P, 512], F32, tag="acc")
                for k in range(KT):
                    nc.tensor.matmul(
                        ps[:],
                        lhsT=w[k][:, m * P:(m + 1) * P],
                        rhs=xT[k][:, tck * 512:(tck + 1) * 512],
                        start=(k == 0),
                        stop=(k == KT - 1),
                    )
                nc.vector.tensor_copy(dst[m][:, tck * 512:(tck + 1) * 512], ps[:])

            def emit_v_chain(t):
                """vext[t][:, h*65:h*65+64] = (x @ Wv^T)[t-block] per head; col 64 = 1."""
                ps = accp.tile([P, 512], F32, tag="acc")
                for k in range(KT):
                    nc.tensor.matmul(
                        ps[:, :GW],
                        lhsT=xT[k][:, t * P:(t + 1) * P],
                        rhs=wvT[k][:],
                        start=(k == 0),
                        stop=(k == KT - 1),
                    )
                vv = vext[t][:].rearrange("p (h e) -> p h e", h=NH)
                pv = ps[:, :GW].rearrange("p (h e) -> p h e", h=NH)
                nc.vector.tensor_copy(vv[:, :, 0:HD], pv)
                nc.any.memset(vv[:, :, HD:HD + 1], 1.0)

            def emit_scores_exp(h, k):
                """P'[h][k] = exp(SCALE * k-block @ q^T)  -- [128 keys, 2048 q] bf16.

                Two 1024-wide halves on a double-buffered PSUM pool so the
                next half's matmuls overlap this half's exp (keeps ACT and
                PE both busy)."""
                m, off = h // 2, (h % 2) * HD
                pp = pexp.tile([P, L], BF16, tag="pp")
                for half in range(2):
                    ps = sps.tile([P, 1024], F32, tag="sc", name=f"sc{h}_{k}_{half}")
                    for q in range(2):
                        qg = half * 2 + q
                        nc.tensor.matmul(
                            ps[:, q * 512:(q + 1) * 512],
                            lhsT=kTt[m][off:off + HD, k * P:(k + 1) * P],
                            rhs=qT[m][off:off + HD, qg * 512:(qg + 1) * 512],
                            start=True,
                            stop=True,
                        )
                    nc.scalar.activation(
                        pp[:, half * 1024:(half + 1) * 1024], ps[:], EXP, scale=SCALE
                    )
                return pp

            def emit_pv_part(h, q, pptiles, ov, k0, k1):
                """Partial PV accumulation over key-tiles [k0, k1)."""
                if ov is None:
                    ov = accp.tile([HD + 1, 512], F32, tag="acc",
                                   name=f"ov{h}_{q}_{k0}")
                for k in range(k0, k1):
                    nc.tensor.matmul(
                        ov[:],
                        lhsT=vext[k][:, h * (HD + 1):(h + 1) * (HD + 1)],
                        rhs=pptiles[k][:, q * 512:(q + 1) * 512],
                        start=(k == 0),
                        stop=(k == TBLK - 1),
                    )
                return ov

            def emit_oe(ov, act=False):
                oe = oep.tile([HD + 1, 512], BF16, tag="oe")
                if act:
                    nc.scalar.copy(oe[0:HD, :], ov[0:HD, :])
                else:
                    nc.vector.tensor_copy(oe[0:HD, :], ov[0:HD, :])
                return oe

            def emit_norm(h, q, ov, oe):
                """aoT[h-rows, q-chunk] = oe[d, q] * (1/sums)[q] (broadcast over d).

                The reciprocal row is broadcast across partitions with a K=1
                matmul against a ones column, then one DVE multiply."""
                m, off = h // 2, (h % 2) * HD
                srow = srp.tile([1, 512], F32, tag="s")
                nc.vector.tensor_copy(srow[:], ov[HD:HD + 1, :])
                rr = rcpp.tile([1, 512], F32, tag="r")
                nc.vector.reciprocal_approx_fast(rr[:], srow[:])
                rrb = rcpp.tile([1, 512], BF16, tag="rb")
                nc.vector.tensor_copy(rrb[:], rr[:])
                br = accp.tile([HD, 512], F32, tag="acc", name=f"br{h}_{q}")
                nc.tensor.matmul(br[:], lhsT=ones64[:], rhs=rrb[:], start=True, stop=True)
                nc.vector.tensor_mul(
                    aoT[m][off:off + HD, q * 512:(q + 1) * 512],
                    oe[0:HD, :],
                    br[:],
                )

            def emit_oproj(t, evict_act=False, split_dma=False):
                """out[t-block] = ao @ W_o[:, gslice]^T  (partial; host sums groups).

                The two 512-col halves evict on different engines (ACT + DVE)
                so they drain in parallel; each half DMAs out as soon as it is
                evicted (row-split for the last tiles to spread queues)."""
                ob = osbp.tile([P, D], BF16, tag="ob")
                for oc in range(2):
                    ps = accp.tile([P, 512], F32, tag="acc")
                    for i in range(GW // P):
                        nc.tensor.matmul(
                            ps[:],
                            lhsT=aoT[i][:, t * P:(t + 1) * P],
                            rhs=woT[i][:, oc * 512:(oc + 1) * 512],
                            start=(i == 0),
                            stop=(i == GW // P - 1),
                        )
                    if evict_act and oc == 0:
                        nc.scalar.copy(ob[:, oc * 512:(oc + 1) * 512], ps[:])
                    else:
                        nc.vector.tensor_copy(ob[:, oc * 512:(oc + 1) * 512], ps[:])
                    if split_dma:
                        for g in range(2):
                            nc.sync.dma_start(
                                out_d[t * P + g * 64:t * P + (g + 1) * 64,
                                      oc * 512:(oc + 1) * 512],
                                ob[g * 64:(g + 1) * 64, oc * 512:(oc + 1) * 512],
                            )
                    else:
                        nc.sync.dma_start(
                            out_d[t * P:(t + 1) * P, oc * 512:(oc + 1) * 512],
                            ob[:, oc * 512:(oc + 1) * 512],
                        )

            # ---- emission schedule ----
            # q/k chains needed by the first scores: all of q(m=0) and the
            # first column-chunk of k(m=0).
            for tcx in range(QC):
                emit_proj_chain(qT, wqT, 0, tcx)
            emit_proj_chain(kTt, wkT, 0, 0)

            # Remaining projection work spread across sections as PE fillers.
            # All v chains must land in section 0: the full PV(0) chain at
            # section-1 kt 0 reads every vext tile.
            fillers = {0: [], 1: [], 2: [], 3: []}
            for tcx in range(1, QC):
                fillers[0].append(lambda tcx=tcx: emit_proj_chain(kTt, wkT, 0, tcx))
            for t in range(TBLK):
                fillers[0].append(lambda t=t: emit_v_chain(t))
            for tcx in range(QC):
                fillers[1].append(lambda tcx=tcx: emit_proj_chain(qT, wqT, 1, tcx))
            for tcx in range(QC):
                fillers[1].append(lambda tcx=tcx: emit_proj_chain(kTt, wkT, 1, tcx))

            pp_prev = None   # P' tiles of head h-1 (being consumed by PV/norm)
            pp_cur = []      # P' tiles of head h (being produced)
            for h in range(NH):
                ovs = [None] * QC
                oes = [None] * QC
                fi = 0
                for k in range(TBLK):
                    # scores first: keeps ACT fed while the PE then runs the
                    # long dense block for this kt.
                    pp_cur.append(emit_scores_exp(h, k))
                    if h == 0:
                        if fi < len(fillers[0]):
                            fillers[0][fi]()
                            fi += 1
                    elif k in (0, 3, 6, 9):
                        # One FULL 16-MM PV chain of head h-1: ~3.4us of
                        # back-to-back matmuls with no semaphore waits -- one
                        # complete HAM busy-window, flipping the PE clock to
                        # 2.4GHz.  With 25 P' slots, exp(h, k) reuses the slot
                        # of pp(h-1, k-9), freed by the last chain (k=9) as it
                        # reads key-tile k-9 -- always in time.
                        q = k // 3
                        ovs[q] = emit_pv_part(h - 1, q, pp_prev, None, 0, TBLK)
                    elif k in (1, 4, 7, 10):
                        q = (k - 1) // 3
                        oes[q] = emit_oe(ovs[q])
                        emit_norm(h - 1, q, ovs[q], oes[q])
                    elif fi < len(fillers[h]):
                        fillers[h][fi]()
                        fi += 1
                for f in fillers[h][fi:]:  # leftovers
                    f()
                pp_prev = pp_cur
                pp_cur = []

            # Tail: PV/norm for the last head + output projection.  With the
            # 2-slot PSUM accumulator pool at most one PV chain is live at a
            # time; O-groups follow their q-chunk's norm.
            h3 = NH - 1
            ov = emit_pv_part(h3, 0, pp_prev, None, 0, TBLK)
            oe = emit_oe(ov, act=True)
            emit_norm(h3, 0, ov, oe)
            ov = emit_pv_part(h3, 1, pp_prev, None, 0, TBLK)
            emit_oproj(0, evict_act=True)
            emit_oproj(1, evict_act=True)
            oe = emit_oe(ov, act=True)
            emit_norm(h3, 1, ov, oe)
            ov = emit_pv_part(h3, 2, pp_prev, None, 0, TBLK)
            emit_oproj(2, evict_act=True)
            emit_oproj(3, evict_act=True)
            emit_oproj(4, evict_act=True)
            emit_oproj(5, evict_act=True)
            oe = emit_oe(ov, act=True)
            emit_norm(h3, 2, ov, oe)
            ov = emit_pv_part(h3, 3, pp_prev, None, 0, TBLK)
            emit_oproj(6, evict_act=True)
            emit_oproj(7, evict_act=True)
            emit_oproj(8, evict_act=True)
            emit_oproj(9, evict_act=True)
            oe = emit_oe(ov, act=True)
            emit_norm(h3, 3, ov, oe)
            emit_oproj(10, evict_act=True)
            emit_oproj(11, evict_act=True)
            for t in range(12, TBLK):
                emit_oproj(t, evict_act=True, split_dma=True)
    nc.compile()
    return nc


_NC = None


def _get_nc():
    global _NC
    if _NC is None:
        _NC = _build()
    return _NC


def _shard(inputs):
    x = np.asarray(inputs["x"], dtype=np.float32)
    W_q = np.asarray(inputs["W_q"], dtype=np.float32)
    W_k = np.asarray(inputs["W_k"], dtype=np.float32)
    W_v = np.asarray(inputs["W_v"], dtype=np.float32)
    W_o = np.asarray(inputs["W_o"], dtype=np.float32)
    bf = ml_dtypes.bfloat16
    in_maps = []
    for core in range(8):
        b, g = core // 4, core % 4
        sl = slice(g * GW, (g + 1) * GW)
        in_maps.append({
            "xT": np.ascontiguousarray(x[b].T).astype(bf),
            "wqT": np.ascontiguousarray(W_q[sl, :].T).astype(bf),
            "wkT": np.ascontiguousarray(W_k[sl, :].T).astype(bf),
            "wvT": np.ascontiguousarray(W_v[sl, :].T).astype(bf),
            "woT": np.ascontiguousarray(W_o[:, sl].T).astype(bf),
        })
    return in_maps


def _run(inputs, trace=False):
    nc = _get_nc()
    in_maps = _shard(inputs)
    res = run_bass_kernel_spmd(nc, in_maps, core_ids=list(range(8)), trace=trace)
    out = np.zeros((B, L, D), dtype=np.float32)
    for core in range(8):
        out[core // 4] += res.results[core]["out"].astype(np.float32)
    return out, res


def kernel(**inputs) -> np.ndarray:
    out, _ = _run(inputs, trace=False)
    return out



# revision 3
# speedup vs baseline: 1.0718x; 1.0718x over previous
"""Trainium2 Bass kernel for multi-head attention (B=2, L=2048, D=1024, H=16).

Sharding: 8 cores = 2 (batch) x 4 (head-groups of 4 heads).  Each core
computes q/k/v projections for its 4 heads, softmax attention, and a
partial output projection against its 256 columns of W_o.  The all-reduce
of the 4 partials per batch happens on the host (free).

All matmuls run in bf16 with fp32 PSUM accumulation.  Softmax skips the
max-subtraction (scores are ~N(0, 1/3); exp is safely in range).

v1: scores matmuls are row-tiled in PAIRS — the K=64 contraction only
fills half the PE array, so two independent score MMs run concurrently
in row groups {0,1} and {2,3} (measured ~2x on HW; see tile_position).
This needs each head's kT replicated into both partition halves (kd)
and q chunks {0,2}/{1,3} staged in halves (qd), built with SBUF->SBUF
DMAs off the projection evictions.
"""

import sys

if "/opt/trn_rl_repo" not in sys.path:
    sys.path.insert(0, "/opt/trn_rl_repo")

import numpy as np
import ml_dtypes

import concourse.bass as bass
import concourse.mybir as mybir
import concourse.tile as tile
from concourse import bacc
from concourse.bass_utils import run_bass_kernel_spmd

B, L, D, H = 2, 2048, 1024, 16
HD = D // H          # 64 head dim
NH = 4               # heads per core
GW = NH * HD         # 256 group width
SCALE = (H / D) ** 0.5  # 1/8
P = 128
KT = D // P          # 8 contraction tiles over D
TBLK = L // P        # 16 token blocks of 128
QC = L // 512        # 4 query chunks of 512
BF16 = mybir.dt.bfloat16
F32 = mybir.dt.float32
EXP = mybir.ActivationFunctionType.Exp

PEXP_BUFS = 25       # P' slots: see v9 slot math (full chains at k=0,3,6,9)


def _build_real():
    nc = bacc.Bacc(None, target_bir_lowering=False, debug=False)

    xT_d = nc.dram_tensor("xT", (D, L), BF16, kind="ExternalInput")
    wqT_d = nc.dram_tensor("wqT", (D, GW), BF16, kind="ExternalInput")
    wkT_d = nc.dram_tensor("wkT", (D, GW), BF16, kind="ExternalInput")
    wvT_d = nc.dram_tensor("wvT", (D, GW), BF16, kind="ExternalInput")
    woT_d = nc.dram_tensor("woT", (GW, D), BF16, kind="ExternalInput")
    out_d = nc.dram_tensor("out", (L, D), BF16, kind="ExternalOutput")

    with tile.TileContext(nc) as tc:
        with (
            tc.tile_pool(name="persist", bufs=1) as pers,
            tc.tile_pool(name="pexp", bufs=PEXP_BUFS) as pexp,
            tc.tile_pool(name="qksc", bufs=3) as qksc,
            tc.tile_pool(name="oeT", bufs=2) as oep,
            tc.tile_pool(name="rcp", bufs=2) as rcpp,
            tc.tile_pool(name="srow", bufs=2) as srp,
            tc.tile_pool(name="osb", bufs=2) as osbp,
            tc.tile_pool(name="spsum", bufs=3, space="PSUM") as sps,
            tc.tile_pool(name="accp", bufs=2, space="PSUM") as accp,
        ):
            # ---- persistent SBUF tensors ----
            xT = [pers.tile([P, L], BF16, tag=f"xT{k}", name=f"xT{k}") for k in range(KT)]
            wqT = [pers.tile([P, GW], BF16, tag=f"wqT{k}", name=f"wqT{k}") for k in range(KT)]
            wkT = [pers.tile([P, GW], BF16, tag=f"wkT{k}", name=f"wkT{k}") for k in range(KT)]
            wvT = [pers.tile([P, GW], BF16, tag=f"wvT{k}", name=f"wvT{k}") for k in range(KT)]
            woT = [pers.tile([P, D], BF16, tag=f"woT{i}", name=f"woT{i}") for i in range(GW // P)]
            # qd[h]: [0:64] = q chunks {0,2}, [64:128] = chunks {1,3} (512 cols each)
            qd = [pers.tile([P, 1024], BF16, tag=f"qd{h}", name=f"qd{h}") for h in range(NH)]
            # kd[h]: head h's kT replicated into both partition halves
            kd = [pers.tile([P, L], BF16, tag=f"kd{h}", name=f"kd{h}") for h in range(NH)]
            vext = [pers.tile([P, NH * (HD + 1)], BF16, tag=f"vx{t}", name=f"vx{t}") for t in range(TBLK)]
            aoT = [pers.tile([P, L], BF16, tag=f"aoT{m}", name=f"aoT{m}") for m in range(GW // P)]
            ones64 = pers.tile([1, HD], BF16, tag="ones64")
            nc.any.memset(ones64[:], 1.0)
            warm = pers.tile([1, 2], BF16, tag="warm")
            nc.scalar.activation(warm[:], ones64[:, 0:2], EXP)  # preload exp table

            for k in range(KT):
                nc.sync.dma_start(xT[k][:], xT_d[k * P:(k + 1) * P, :])
                nc.sync.dma_start(wqT[k][:], wqT_d[k * P:(k + 1) * P, :])
                nc.sync.dma_start(wkT[k][:], wkT_d[k * P:(k + 1) * P, :])
            for k in range(KT):
                nc.sync.dma_start(wvT[k][:], wvT_d[k * P:(k + 1) * P, :])
            for i in range(GW // P):
                nc.sync.dma_start(woT[i][:], woT_d[i * P:(i + 1) * P, :])

            # ---- helper emitters ----
            def emit_q_chain(m, tck):
                """q projection for head pair m, chunk tck -> staged into qd.

                qd[h][half, (tck//2)*512] where half = 64*(tck%2)."""
                ps = accp.tile([P, 512], F32, tag="acc")
                for k in range(KT):
                    nc.tensor.matmul(
                        ps[:],
                        lhsT=wqT[k][:, m * P:(m + 1) * P],
                        rhs=xT[k][:, tck * 512:(tck + 1) * 512],
                        start=(k == 0),
                        stop=(k == KT - 1),
                    )
                s = qksc.tile([P, 512], BF16, tag="qk")
                nc.vector.tensor_copy(s[:], ps[:])
                po = (tck % 2) * HD
                co = (tck // 2) * 512
                nc.sync.dma_start(qd[2 * m][po:po + HD, co:co + 512], s[0:HD, :])
                nc.sync.dma_start(qd[2 * m + 1][po:po + HD, co:co + 512], s[HD:P, :])

            def emit_k_chain(m, tck):
                """k projection for head pair m, chunk tck -> kd both halves."""
                ps = accp.tile([P, 512], F32, tag="acc")
                for k in range(KT):
                    nc.tensor.matmul(
                        ps[:],
                        lhsT=wkT[k][:, m * P:(m + 1) * P],
                        rhs=xT[k][:, tck * 512:(tck + 1) * 512],
                        start=(k == 0),
                        stop=(k == KT - 1),
                    )
                s = qksc.tile([P, 512], BF16, tag="qk")
                nc.vector.tensor_copy(s[:], ps[:])
                co = tck * 512
                nc.sync.dma_start(kd[2 * m][0:HD, co:co + 512], s[0:HD, :])
                nc.sync.dma_start(kd[2 * m][HD:P, co:co + 512], s[0:HD, :])
                nc.sync.dma_start(kd[2 * m + 1][0:HD, co:co + 512], s[HD:P, :])
                nc.sync.dma_start(kd[2 * m + 1][HD:P, co:co + 512], s[HD:P, :])

            def emit_v_chain(t):
                """vext[t][:, h*65:h*65+64] = (x @ Wv^T)[t-block] per head; col 64 = 1."""
                ps = accp.tile([P, 512], F32, tag="acc")
                for k in range(KT):
                    nc.tensor.matmul(
                        ps[:, :GW],
                        lhsT=xT[k][:, t * P:(t + 1) * P],
                        rhs=wvT[k][:],
                        start=(k == 0),
                        stop=(k == KT - 1),
                    )
                vv = vext[t][:].rearrange("p (h e) -> p h e", h=NH)
                pv = ps[:, :GW].rearrange("p (h e) -> p h e", h=NH)
                nc.vector.tensor_copy(vv[:, :, 0:HD], pv)
                nc.any.memset(vv[:, :, HD:HD + 1], 1.0)

            def emit_scores_exp(h, k):
                """P'[h][k] = exp(SCALE * k-block @ q^T)  -- [128 keys, 2048 q] bf16.

                Row-tiled pairs: the K=64 contraction uses half the PE rows,
                so the two q-chunks of each half run CONCURRENTLY in row
                groups {0,1} (partitions 0:64) and {2,3} (partitions 64:128).
                kd has the k-block in both halves; qd stages chunk 2c in the
                low half and 2c+1 in the high half."""
                pp = pexp.tile([P, L], BF16, tag="pp")
                for half in range(2):
                    ps = sps.tile([P, 1024], F32, tag="sc", name=f"sc{h}_{k}_{half}")
                    for q in range(2):
                        po = q * HD
                        nc.tensor.matmul(
                            ps[:, q * 512:(q + 1) * 512],
                            lhsT=kd[h][po:po + HD, k * P:(k + 1) * P],
                            rhs=qd[h][po:po + HD, half * 512:(half + 1) * 512],
                            start=True,
                            stop=True,
                        )
                    nc.scalar.activation(
                        pp[:, half * 1024:(half + 1) * 1024], ps[:], EXP, scale=SCALE
                    )
                return pp

            def emit_pv_part(h, q, pptiles, ov, k0, k1):
                """Partial PV accumulation over key-tiles [k0, k1)."""
                if ov is None:
                    ov = accp.tile([HD + 1, 512], F32, tag="acc",
                                   name=f"ov{h}_{q}_{k0}")
                for k in range(k0, k1):
                    nc.tensor.matmul(
                        ov[:],
                        lhsT=vext[k][:, h * (HD + 1):(h + 1) * (HD + 1)],
                        rhs=pptiles[k][:, q * 512:(q + 1) * 512],
                        start=(k == 0),
                        stop=(k == TBLK - 1),
                    )
                return ov

            def emit_oe(ov, act=False):
                oe = oep.tile([HD + 1, 512], BF16, tag="oe")
                if act:
                    nc.scalar.copy(oe[0:HD, :], ov[0:HD, :])
                else:
                    nc.vector.tensor_copy(oe[0:HD, :], ov[0:HD, :])
                return oe

            def emit_norm(h, q, ov, oe):
                """aoT[h-rows, q-chunk] = oe[d, q] * (1/sums)[q] (broadcast over d).

                The reciprocal row is broadcast across partitions with a K=1
                matmul against a ones column, then one DVE multiply."""
                m, off = h // 2, (h % 2) * HD
                srow = srp.tile([1, 512], F32, tag="s")
                nc.vector.tensor_copy(srow[:], ov[HD:HD + 1, :])
                rr = rcpp.tile([1, 512], F32, tag="r")
                nc.vector.reciprocal_approx_fast(rr[:], srow[:])
                rrb = rcpp.tile([1, 512], BF16, tag="rb")
                nc.vector.tensor_copy(rrb[:], rr[:])
                br = accp.tile([HD, 512], F32, tag="acc", name=f"br{h}_{q}")
                nc.tensor.matmul(br[:], lhsT=ones64[:], rhs=rrb[:], start=True, stop=True)
                nc.vector.tensor_mul(
                    aoT[m][off:off + HD, q * 512:(q + 1) * 512],
                    oe[0:HD, :],
                    br[:],
                )

            def emit_oproj(t, evict_act=False, split_dma=False):
                """out[t-block] = ao @ W_o[:, gslice]^T  (partial; host sums groups).

                The two 512-col halves evict on different engines (ACT + DVE)
                so they drain in parallel; each half DMAs out as soon as it is
                evicted (row-split for the last tiles to spread queues)."""
                ob = osbp.tile([P, D], BF16, tag="ob")
                for oc in range(2):
                    ps = accp.tile([P, 512], F32, tag="acc")
                    for i in range(GW // P):
                        nc.tensor.matmul(
                            ps[:],
                            lhsT=aoT[i][:, t * P:(t + 1) * P],
                            rhs=woT[i][:, oc * 512:(oc + 1) * 512],
                            start=(i == 0),
                            stop=(i == GW // P - 1),
                        )
                    if evict_act and oc == 0:
                        nc.scalar.copy(ob[:, oc * 512:(oc + 1) * 512], ps[:])
                    else:
                        nc.vector.tensor_copy(ob[:, oc * 512:(oc + 1) * 512], ps[:])
                    if split_dma:
                        for g in range(2):
                            nc.sync.dma_start(
                                out_d[t * P + g * 64:t * P + (g + 1) * 64,
                                      oc * 512:(oc + 1) * 512],
                                ob[g * 64:(g + 1) * 64, oc * 512:(oc + 1) * 512],
                            )
                    else:
                        nc.sync.dma_start(
                            out_d[t * P:(t + 1) * P, oc * 512:(oc + 1) * 512],
                            ob[:, oc * 512:(oc + 1) * 512],
                        )

            # ---- emission schedule ----
            # q/k chains needed by the first scores: all of q(m=0) and the
            # first column-chunk of k(m=0).
            for tcx in range(QC):
                emit_q_chain(0, tcx)
            emit_k_chain(0, 0)

            # Remaining projection work spread across sections as PE fillers.
            # All v chains must land in section 0: the full PV(0) chain at
            # section-1 kt 0 reads every vext tile.
            fillers = {0: [], 1: [], 2: [], 3: []}
            for tcx in range(1, QC):
                fillers[0].append(lambda tcx=tcx: emit_k_chain(0, tcx))
            for t in range(TBLK):
                fillers[0].append(lambda t=t: emit_v_chain(t))
            for tcx in range(QC):
                fillers[1].append(lambda tcx=tcx: emit_q_chain(1, tcx))
            for tcx in range(QC):
                fillers[1].append(lambda tcx=tcx: emit_k_chain(1, tcx))

            pp_prev = None   # P' tiles of head h-1 (being consumed by PV/norm)
            pp_cur = []      # P' tiles of head h (being produced)
            for h in range(NH):
                ovs = [None] * QC
                oes = [None] * QC
                fi = 0
                for k in range(TBLK):
                    # scores first: keeps ACT fed while the PE then runs the
                    # long dense block for this kt.
                    pp_cur.append(emit_scores_exp(h, k))
                    if h == 0:
                        if fi < len(fillers[0]):
                            fillers[0][fi]()
                            fi += 1
                    elif k in (0, 3, 6, 9):
                        # One FULL 16-MM PV chain of head h-1: ~3.4us of
                        # back-to-back matmuls with no semaphore waits -- one
                        # complete HAM busy-window, flipping the PE clock to
                        # 2.4GHz.  With 25 P' slots, exp(h, k) reuses the slot
                        # of pp(h-1, k-9), freed by the last chain (k=9) as it
                        # reads key-tile k-9 -- always in time.
                        q = k // 3
                        ovs[q] = emit_pv_part(h - 1, q, pp_prev, None, 0, TBLK)
                    elif k in (1, 4, 7, 10):
                        q = (k - 1) // 3
                        oes[q] = emit_oe(ovs[q])
                        emit_norm(h - 1, q, ovs[q], oes[q])
                    elif fi < len(fillers[h]):
                        fillers[h][fi]()
                        fi += 1
                for f in fillers[h][fi:]:  # leftovers
                    f()
                pp_prev = pp_cur
                pp_cur = []

            # Tail: PV/norm for the last head + output projection.  With the
            # 2-slot PSUM accumulator pool at most one PV chain is live at a
            # time; O-groups follow their q-chunk's norm.
            h3 = NH - 1
            ov = emit_pv_part(h3, 0, pp_prev, None, 0, TBLK)
            oe = emit_oe(ov, act=True)
            emit_norm(h3, 0, ov, oe)
            ov = emit_pv_part(h3, 1, pp_prev, None, 0, TBLK)
            emit_oproj(0, evict_act=True)
            emit_oproj(1, evict_act=True)
            oe = emit_oe(ov, act=True)
            emit_norm(h3, 1, ov, oe)
            ov = emit_pv_part(h3, 2, pp_prev, None, 0, TBLK)
            emit_oproj(2, evict_act=True)
            emit_oproj(3, evict_act=True)
            emit_oproj(4, evict_act=True)
            emit_oproj(5, evict_act=True)
            oe = emit_oe(ov, act=True)
            emit_norm(h3, 2, ov, oe)
            ov = emit_pv_part(h3, 3, pp_prev, None, 0, TBLK)
            emit_oproj(6, evict_act=True)
            emit_oproj(7, evict_act=True)
            emit_oproj(8, evict_act=True)
            emit_oproj(9, evict_act=True)
            oe = emit_oe(ov, act=True)
            emit_norm(h3, 3, ov, oe)
            emit_oproj(10, evict_act=True)
            emit_oproj(11, evict_act=True)
            for t in range(12, TBLK):
                emit_oproj(t, evict_act=True, split_dma=True)
    nc.compile()
    return nc


_NC = None


def _get_nc():
    global _NC
    if _NC is None:
        _NC = _build_real()
    return _NC


def _shard(inputs):
    x = np.asarray(inputs["x"], dtype=np.float32)
    W_q = np.asarray(inputs["W_q"], dtype=np.float32)
    W_k = np.asarray(inputs["W_k"], dtype=np.float32)
    W_v = np.asarray(inputs["W_v"], dtype=np.float32)
    W_o = np.asarray(inputs["W_o"], dtype=np.float32)
    bf = ml_dtypes.bfloat16
    in_maps = []
    for core in range(8):
        b, g = core // 4, core % 4
        sl = slice(g * GW, (g + 1) * GW)
        in_maps.append({
            "xT": np.ascontiguousarray(x[b].T).astype(bf),
            "wqT": np.ascontiguousarray(W_q[sl, :].T).astype(bf),
            "wkT": np.ascontiguousarray(W_k[sl, :].T).astype(bf),
            "wvT": np.ascontiguousarray(W_v[sl, :].T).astype(bf),
            "woT": np.ascontiguousarray(W_o[:, sl].T).astype(bf),
        })
    return in_maps


def _run(inputs, trace=False):
    nc = _get_nc()
    in_maps = _shard(inputs)
    res = run_bass_kernel_spmd(nc, in_maps, core_ids=list(range(8)), trace=trace)
    out = np.zeros((B, L, D), dtype=np.float32)
    for core in range(8):
        out[core // 4] += res.results[core]["out"].astype(np.float32)
    return out, res


def kernel(**inputs) -> np.ndarray:
    out, _ = _run(inputs, trace=False)
    return out


# revision 4
# speedup vs baseline: 1.4018x; 1.3078x over previous
"""Trainium2 Bass kernel for multi-head attention (B=2, L=2048, D=1024, H=16).

Sharding: 8 cores = 2 (batch) x 4 (head-groups of 4 heads).  Each core
computes q/k/v projections for its 4 heads, softmax attention, and a
partial output projection against its 256 columns of W_o.  The all-reduce
of the 4 partials per batch happens on the host (free).

All matmuls run in bf16 with fp32 PSUM accumulation.  Softmax skips the
max-subtraction (scores are ~N(0, 1/3); exp is safely in range).

v1: scores matmuls are row-tiled in PAIRS — the K=64 contraction only
fills half the PE array, so two independent score MMs run concurrently
in row groups {0,1} and {2,3}.  Needs each head's kT replicated into
both partition halves (kd) and q chunks {0,2}/{1,3} staged in halves
(qd), built with SBUF->SBUF DMAs off the projection evictions.

v2: the steady state is ACT-bound (exp issue ~1.2us per [128,1024]
half).  PV chains are split into 4-MM segments interleaved between the
score pairs so the PE produces score tiles at the exp cadence instead
of bursting: chunks run in pairs (0,1 over steps 0-7; 2,3 over 8-15),
alternating segments each step.  This keeps the 25-slot P' budget
(peak 24) and needs 2 PV PSUM accumulators.  PSUM: 2x[128,1024] score
tiles + 2 PV banks + 2 fill banks = 8.
"""

import sys

if "/opt/trn_rl_repo" not in sys.path:
    sys.path.insert(0, "/opt/trn_rl_repo")

import numpy as np
import ml_dtypes

import concourse.bass as bass
import concourse.mybir as mybir
import concourse.tile as tile
from concourse import bacc
from concourse.bass_utils import run_bass_kernel_spmd

B, L, D, H = 2, 2048, 1024, 16
HD = D // H          # 64 head dim
NH = 4               # heads per core
GW = NH * HD         # 256 group width
SCALE = (H / D) ** 0.5  # 1/8
P = 128
KT = D // P          # 8 contraction tiles over D
TBLK = L // P        # 16 token blocks of 128
QC = L // 512        # 4 query chunks of 512
BF16 = mybir.dt.bfloat16
F32 = mybir.dt.float32
EXP = mybir.ActivationFunctionType.Exp

PEXP_BUFS = 25


def _build():
    nc = bacc.Bacc(None, target_bir_lowering=False, debug=False)

    xT_d = nc.dram_tensor("xT", (D, L), BF16, kind="ExternalInput")
    wqT_d = nc.dram_tensor("wqT", (D, GW), BF16, kind="ExternalInput")
    wkT_d = nc.dram_tensor("wkT", (D, GW), BF16, kind="ExternalInput")
    wvT_d = nc.dram_tensor("wvT", (D, GW), BF16, kind="ExternalInput")
    woT_d = nc.dram_tensor("woT", (GW, D), BF16, kind="ExternalInput")
    out_d = nc.dram_tensor("out", (L, D), BF16, kind="ExternalOutput")

    with tile.TileContext(nc) as tc:
        with (
            tc.tile_pool(name="persist", bufs=1) as pers,
            tc.tile_pool(name="pexp", bufs=PEXP_BUFS) as pexp,
            tc.tile_pool(name="qksc", bufs=3) as qksc,
            tc.tile_pool(name="oeT", bufs=2) as oep,
            tc.tile_pool(name="rcp", bufs=2) as rcpp,
            tc.tile_pool(name="srow", bufs=2) as srp,
            tc.tile_pool(name="osb", bufs=2) as osbp,
            tc.tile_pool(name="spsum", bufs=2, space="PSUM") as sps,
            tc.tile_pool(name="accp", bufs=2, space="PSUM") as accp,
        ):
            # ---- persistent SBUF tensors ----
            xT = [pers.tile([P, L], BF16, tag=f"xT{k}", name=f"xT{k}") for k in range(KT)]
            wqT = [pers.tile([P, GW], BF16, tag=f"wqT{k}", name=f"wqT{k}") for k in range(KT)]
            wkT = [pers.tile([P, GW], BF16, tag=f"wkT{k}", name=f"wkT{k}") for k in range(KT)]
            wvT = [pers.tile([P, GW], BF16, tag=f"wvT{k}", name=f"wvT{k}") for k in range(KT)]
            woT = [pers.tile([P, D], BF16, tag=f"woT{i}", name=f"woT{i}") for i in range(GW // P)]
            # qd[h]: [0:64] = q chunks {0,2}, [64:128] = chunks {1,3} (512 cols each)
            qd = [pers.tile([P, 1024], BF16, tag=f"qd{h}", name=f"qd{h}") for h in range(NH)]
            # kd[h]: head h's kT replicated into both partition halves
            kd = [pers.tile([P, L], BF16, tag=f"kd{h}", name=f"kd{h}") for h in range(NH)]
            vext = [pers.tile([P, NH * (HD + 1)], BF16, tag=f"vx{t}", name=f"vx{t}") for t in range(TBLK)]
            aoT = [pers.tile([P, L], BF16, tag=f"aoT{m}", name=f"aoT{m}") for m in range(GW // P)]
            ones64 = pers.tile([1, HD], BF16, tag="ones64")
            nc.any.memset(ones64[:], 1.0)
            warm = pers.tile([1, 2], BF16, tag="warm")
            nc.scalar.activation(warm[:], ones64[:, 0:2], EXP)  # preload exp table

            for k in range(KT):
                nc.sync.dma_start(xT[k][:], xT_d[k * P:(k + 1) * P, :])
                nc.sync.dma_start(wqT[k][:], wqT_d[k * P:(k + 1) * P, :])
                nc.sync.dma_start(wkT[k][:], wkT_d[k * P:(k + 1) * P, :])
            for k in range(KT):
                nc.sync.dma_start(wvT[k][:], wvT_d[k * P:(k + 1) * P, :])
            for i in range(GW // P):
                nc.sync.dma_start(woT[i][:], woT_d[i * P:(i + 1) * P, :])

            # ---- helper emitters ----
            def emit_q_chain(m, tck):
                """q projection for head pair m, chunk tck -> staged into qd."""
                ps = accp.tile([P, 512], F32, tag="fill")
                for k in range(KT):
                    nc.tensor.matmul(
                        ps[:],
                        lhsT=wqT[k][:, m * P:(m + 1) * P],
                        rhs=xT[k][:, tck * 512:(tck + 1) * 512],
                        start=(k == 0),
                        stop=(k == KT - 1),
                    )
                s = qksc.tile([P, 512], BF16, tag="qk")
                nc.vector.tensor_copy(s[:], ps[:])
                po = (tck % 2) * HD
                co = (tck // 2) * 512
                nc.sync.dma_start(qd[2 * m][po:po + HD, co:co + 512], s[0:HD, :])
                nc.sync.dma_start(qd[2 * m + 1][po:po + HD, co:co + 512], s[HD:P, :])

            def emit_k_chain(m, tck):
                """k projection for head pair m, chunk tck -> kd both halves."""
                ps = accp.tile([P, 512], F32, tag="fill")
                for k in range(KT):
                    nc.tensor.matmul(
                        ps[:],
                        lhsT=wkT[k][:, m * P:(m + 1) * P],
                        rhs=xT[k][:, tck * 512:(tck + 1) * 512],
                        start=(k == 0),
                        stop=(k == KT - 1),
                    )
                s = qksc.tile([P, 512], BF16, tag="qk")
                nc.vector.tensor_copy(s[:], ps[:])
                co = tck * 512
                nc.sync.dma_start(kd[2 * m][0:HD, co:co + 512], s[0:HD, :])
                nc.sync.dma_start(kd[2 * m][HD:P, co:co + 512], s[0:HD, :])
                nc.sync.dma_start(kd[2 * m + 1][0:HD, co:co + 512], s[HD:P, :])
                nc.sync.dma_start(kd[2 * m + 1][HD:P, co:co + 512], s[HD:P, :])

            def emit_v_chain(t):
                """vext[t][:, h*65:h*65+64] = (x @ Wv^T)[t-block] per head; col 64 = 1."""
                ps = accp.tile([P, 512], F32, tag="fill")
                for k in range(KT):
                    nc.tensor.matmul(
                        ps[:, :GW],
                        lhsT=xT[k][:, t * P:(t + 1) * P],
                        rhs=wvT[k][:],
                        start=(k == 0),
                        stop=(k == KT - 1),
                    )
                vv = vext[t][:].rearrange("p (h e) -> p h e", h=NH)
                pv = ps[:, :GW].rearrange("p (h e) -> p h e", h=NH)
                nc.vector.tensor_copy(vv[:, :, 0:HD], pv)
                nc.any.memset(vv[:, :, HD:HD + 1], 1.0)

            def emit_scores_exp(h, k):
                """P'[h][k] = exp(SCALE * k-block @ q^T)  -- [128 keys, 2048 q] bf16.

                Row-tiled pairs: the two q-chunks of each half run
                CONCURRENTLY in PE row groups {0,1} / {2,3}."""
                pp = pexp.tile([P, L], BF16, tag="pp")
                for half in range(2):
                    ps = sps.tile([P, 1024], F32, tag="sc", name=f"sc{h}_{k}_{half}")
                    for q in range(2):
                        po = q * HD
                        nc.tensor.matmul(
                            ps[:, q * 512:(q + 1) * 512],
                            lhsT=kd[h][po:po + HD, k * P:(k + 1) * P],
                            rhs=qd[h][po:po + HD, half * 512:(half + 1) * 512],
                            start=True,
                            stop=True,
                        )
                    nc.scalar.activation(
                        pp[:, half * 1024:(half + 1) * 1024], ps[:], EXP, scale=SCALE
                    )
                return pp

            def emit_pv_seg(h, q, pptiles, ov, k0, k1):
                """PV segment: accumulate key-tiles [k0, k1) for (head, chunk)."""
                if ov is None:
                    ov = accp.tile([HD + 1, 512], F32, tag="pv",
                                   name=f"ov{h}_{q}")
                for k in range(k0, k1):
                    nc.tensor.matmul(
                        ov[:],
                        lhsT=vext[k][:, h * (HD + 1):(h + 1) * (HD + 1)],
                        rhs=pptiles[k][:, q * 512:(q + 1) * 512],
                        start=(k == 0),
                        stop=(k == TBLK - 1),
                    )
                return ov

            def emit_oe(ov, act=False):
                oe = oep.tile([HD + 1, 512], BF16, tag="oe")
                if act:
                    nc.scalar.copy(oe[0:HD, :], ov[0:HD, :])
                else:
                    nc.vector.tensor_copy(oe[0:HD, :], ov[0:HD, :])
                return oe

            def emit_norm(h, q, ov, oe):
                """aoT[h-rows, q-chunk] = oe[d, q] * (1/sums)[q] (broadcast over d)."""
                m, off = h // 2, (h % 2) * HD
                srow = srp.tile([1, 512], F32, tag="s")
                nc.vector.tensor_copy(srow[:], ov[HD:HD + 1, :])
                rr = rcpp.tile([1, 512], F32, tag="r")
                nc.vector.reciprocal_approx_fast(rr[:], srow[:])
                rrb = rcpp.tile([1, 512], BF16, tag="rb")
                nc.vector.tensor_copy(rrb[:], rr[:])
                br = accp.tile([HD, 512], F32, tag="fill", name=f"br{h}_{q}")
                nc.tensor.matmul(br[:], lhsT=ones64[:], rhs=rrb[:], start=True, stop=True)
                nc.vector.tensor_mul(
                    aoT[m][off:off + HD, q * 512:(q + 1) * 512],
                    oe[0:HD, :],
                    br[:],
                )

            def emit_pvnorm(h, q, ovs, act=False):
                oe = emit_oe(ovs[q], act=act)
                emit_norm(h, q, ovs[q], oe)

            def emit_oproj(t, evict_act=False, split_dma=False):
                """out[t-block] = ao @ W_o[:, gslice]^T  (partial; host sums groups)."""
                ob = osbp.tile([P, D], BF16, tag="ob")
                for oc in range(2):
                    ps = accp.tile([P, 512], F32, tag="fill")
                    for i in range(GW // P):
                        nc.tensor.matmul(
                            ps[:],
                            lhsT=aoT[i][:, t * P:(t + 1) * P],
                            rhs=woT[i][:, oc * 512:(oc + 1) * 512],
                            start=(i == 0),
                            stop=(i == GW // P - 1),
                        )
                    if evict_act and oc == 0:
                        nc.scalar.copy(ob[:, oc * 512:(oc + 1) * 512], ps[:])
                    else:
                        nc.vector.tensor_copy(ob[:, oc * 512:(oc + 1) * 512], ps[:])
                    if split_dma:
                        for g in range(2):
                            nc.sync.dma_start(
                                out_d[t * P + g * 64:t * P + (g + 1) * 64,
                                      oc * 512:(oc + 1) * 512],
                                ob[g * 64:(g + 1) * 64, oc * 512:(oc + 1) * 512],
                            )
                    else:
                        nc.sync.dma_start(
                            out_d[t * P:(t + 1) * P, oc * 512:(oc + 1) * 512],
                            ob[:, oc * 512:(oc + 1) * 512],
                        )

            # ---- emission schedule ----
            # q/k chains needed by the first scores: all of q(m=0) and the
            # first column-chunk of k(m=0).
            for tcx in range(QC):
                emit_q_chain(0, tcx)
            emit_k_chain(0, 0)

            # Remaining projection work spread across the head iterations as
            # PE fillers.  All v chains must land in head 0: head 1's PV
            # segments read vext from step 0.
            fillers = {0: [], 1: [], 2: [], 3: []}
            for tcx in range(1, QC):
                fillers[0].append(lambda tcx=tcx: emit_k_chain(0, tcx))
            for t in range(TBLK):
                fillers[0].append(lambda t=t: emit_v_chain(t))
            for tcx in range(QC):
                fillers[1].append(lambda tcx=tcx: emit_q_chain(1, tcx))
            for tcx in range(QC):
                fillers[1].append(lambda tcx=tcx: emit_k_chain(1, tcx))

            # Per head iteration: 16 k-steps.  Each step emits (PE order)
            # the PV segment of the previous head, then the score pair +
            # exps -- so P' slots freed by the segment are available to the
            # step's exp, and the PE never bursts >~2us without producing a
            # score tile for ACT.  PV chunk schedule: chunks 0,1 alternate
            # 4-MM segments over steps 0-7; chunks 2,3 over steps 8-15.
            # Chunk chains complete at steps 6,7,14,15; norms follow one
            # step later (chunk 3's norm lands after the loop).
            pp_prev = None
            pp_cur = []
            for h in range(NH):
                hp = h - 1
                ovs = [None] * QC
                fi = 0
                for k in range(TBLK):
                    if h > 0:
                        q = (k // 8) * 2 + (k % 2)
                        seg = (k % 8) // 2
                        ovs[q] = emit_pv_seg(hp, q, pp_prev, ovs[q],
                                             seg * 4, seg * 4 + 4)
                    pp_cur.append(emit_scores_exp(h, k))
                    if h > 0:
                        if k == 7:
                            emit_pvnorm(hp, 0, ovs)
                        elif k == 8:
                            emit_pvnorm(hp, 1, ovs)
                        elif k == 15:
                            emit_pvnorm(hp, 2, ovs)
                    # filler pacing: stay on schedule across the 16 steps
                    nf = len(fillers[h])
                    if nf:
                        tgt = ((k + 1) * nf + TBLK - 1) // TBLK
                        while fi < min(tgt, nf):
                            fillers[h][fi]()
                            fi += 1
                if h > 0:
                    emit_pvnorm(hp, 3, ovs)
                for f in fillers[h][fi:]:
                    f()
                pp_prev = pp_cur
                pp_cur = []

            # ---- tail: PV/norm for head 3 + output projection ----
            h3 = NH - 1
            ovs = [None] * QC
            for seg in range(4):
                ovs[0] = emit_pv_seg(h3, 0, pp_prev, ovs[0], seg * 4, seg * 4 + 4)
                ovs[1] = emit_pv_seg(h3, 1, pp_prev, ovs[1], seg * 4, seg * 4 + 4)
            emit_pvnorm(h3, 0, ovs)          # DVE evict: ACT still on last exps
            emit_pvnorm(h3, 1, ovs)
            for seg in range(4):
                ovs[2] = emit_pv_seg(h3, 2, pp_prev, ovs[2], seg * 4, seg * 4 + 4)
                ovs[3] = emit_pv_seg(h3, 3, pp_prev, ovs[3], seg * 4, seg * 4 + 4)
                emit_oproj(seg, evict_act=True)
            emit_pvnorm(h3, 2, ovs, act=True)
            for t in range(4, 8):
                emit_oproj(t, evict_act=True)
            emit_pvnorm(h3, 3, ovs, act=True)
            for t in range(8, 12):
                emit_oproj(t, evict_act=True)
            for t in range(12, TBLK):
                emit_oproj(t, evict_act=True, split_dma=True)
    nc.compile()
    return nc


_NC = None


def _get_nc():
    global _NC
    if _NC is None:
        _NC = _build()
    return _NC


def _shard(inputs):
    x = np.asarray(inputs["x"], dtype=np.float32)
    W_q = np.asarray(inputs["W_q"], dtype=np.float32)
    W_k = np.asarray(inputs["W_k"], dtype=np.float32)
    W_v = np.asarray(inputs["W_v"], dtype=np.float32)
    W_o = np.asarray(inputs["W_o"], dtype=np.float32)
    bf = ml_dtypes.bfloat16
    in_maps = []
    for core in range(8):
        b, g = core // 4, core % 4
        sl = slice(g * GW, (g + 1) * GW)
        in_maps.append({
            "xT": np.ascontiguousarray(x[b].T).astype(bf),
            "wqT": np.ascontiguousarray(W_q[sl, :].T).astype(bf),
            "wkT": np.ascontiguousarray(W_k[sl, :].T).astype(bf),
            "wvT": np.ascontiguousarray(W_v[sl, :].T).astype(bf),
            "woT": np.ascontiguousarray(W_o[:, sl].T).astype(bf),
        })
    return in_maps


def _run(inputs, trace=False):
    nc = _get_nc()
    in_maps = _shard(inputs)
    res = run_bass_kernel_spmd(nc, in_maps, core_ids=list(range(8)), trace=trace)
    out = np.zeros((B, L, D), dtype=np.float32)
    for core in range(8):
        out[core // 4] += res.results[core]["out"].astype(np.float32)
    return out, res


def kernel(**inputs) -> np.ndarray:
    out, _ = _run(inputs, trace=False)
    return out


# revision 7
# speedup vs baseline: 1.4055x; 1.0027x over previous
"""Trainium2 Bass kernel for multi-head attention (B=2, L=2048, D=1024, H=16).

Sharding: 8 cores = 2 (batch) x 4 (head-groups of 4 heads).  Each core
computes q/k/v projections for its 4 heads, softmax attention, and a
partial output projection against its 256 columns of W_o.  The all-reduce
of the 4 partials per batch happens on the host (free).

All matmuls run in bf16 with fp32 PSUM accumulation.  Softmax skips the
max-subtraction (scores are ~N(0, 1/3); exp is safely in range).

v1: scores matmuls are row-tiled in PAIRS — the K=64 contraction only
fills half the PE array, so two independent score MMs run concurrently
in row groups {0,1} and {2,3}.  Needs each head's kT replicated into
both partition halves (kd) and q chunks {0,2}/{1,3} staged in halves
(qd), built with SBUF->SBUF DMAs off the projection evictions.

v2: the steady state is ACT-bound (exp issue ~1.2us per [128,1024]
half).  PV chains are split into 4-MM segments interleaved between the
score pairs so the PE produces score tiles at the exp cadence instead
of bursting: chunks run in pairs (0,1 over steps 0-7; 2,3 over 8-15),
alternating segments each step.  This keeps the 25-slot P' budget
(peak 24) and needs 2 PV PSUM accumulators.  PSUM: 2x[128,1024] score
tiles + 2 PV banks + 2 fill banks = 8.
"""

import sys

if "/opt/trn_rl_repo" not in sys.path:
    sys.path.insert(0, "/opt/trn_rl_repo")

import numpy as np
import ml_dtypes

import concourse.bass as bass
import concourse.mybir as mybir
import concourse.tile as tile
from concourse import bacc
from concourse.bass_utils import run_bass_kernel_spmd

B, L, D, H = 2, 2048, 1024, 16
HD = D // H          # 64 head dim
NH = 4               # heads per core
GW = NH * HD         # 256 group width
SCALE = (H / D) ** 0.5  # 1/8
P = 128
KT = D // P          # 8 contraction tiles over D
TBLK = L // P        # 16 token blocks of 128
QC = L // 512        # 4 query chunks of 512
BF16 = mybir.dt.bfloat16
F32 = mybir.dt.float32
EXP = mybir.ActivationFunctionType.Exp

PEXP_BUFS = 25


def _build():
    nc = bacc.Bacc(None, target_bir_lowering=False, debug=False)

    xT_d = nc.dram_tensor("xT", (D, L), BF16, kind="ExternalInput")
    wqT_d = nc.dram_tensor("wqT", (D, GW), BF16, kind="ExternalInput")
    wkT_d = nc.dram_tensor("wkT", (D, GW), BF16, kind="ExternalInput")
    wvT_d = nc.dram_tensor("wvT", (D, GW), BF16, kind="ExternalInput")
    woT_d = nc.dram_tensor("woT", (GW, D), BF16, kind="ExternalInput")
    out_d = nc.dram_tensor("out", (L, D), BF16, kind="ExternalOutput")

    with tile.TileContext(nc) as tc:
        with (
            tc.tile_pool(name="persist", bufs=1) as pers,
            tc.tile_pool(name="pexp", bufs=PEXP_BUFS) as pexp,
            tc.tile_pool(name="qksc", bufs=3) as qksc,
            tc.tile_pool(name="oeT", bufs=2) as oep,
            tc.tile_pool(name="rcp", bufs=2) as rcpp,
            tc.tile_pool(name="srow", bufs=2) as srp,
            tc.tile_pool(name="osb", bufs=2) as osbp,
            tc.tile_pool(name="spsum", bufs=2, space="PSUM") as sps,
            tc.tile_pool(name="accp", bufs=2, space="PSUM") as accp,
        ):
            # ---- persistent SBUF tensors ----
            xT = [pers.tile([P, L], BF16, tag=f"xT{k}", name=f"xT{k}") for k in range(KT)]
            wqT = [pers.tile([P, GW], BF16, tag=f"wqT{k}", name=f"wqT{k}") for k in range(KT)]
            wkT = [pers.tile([P, GW], BF16, tag=f"wkT{k}", name=f"wkT{k}") for k in range(KT)]
            wvT = [pers.tile([P, GW], BF16, tag=f"wvT{k}", name=f"wvT{k}") for k in range(KT)]
            woT = [pers.tile([P, D], BF16, tag=f"woT{i}", name=f"woT{i}") for i in range(GW // P)]
            # qd[h]: [0:64] = q chunks {0,2}, [64:128] = chunks {1,3} (512 cols each)
            qd = [pers.tile([P, 1024], BF16, tag=f"qd{h}", name=f"qd{h}") for h in range(NH)]
            # kd[h]: head h's kT replicated into both partition halves
            kd = [pers.tile([P, L], BF16, tag=f"kd{h}", name=f"kd{h}") for h in range(NH)]
            vext = [pers.tile([P, NH * (HD + 1)], BF16, tag=f"vx{t}", name=f"vx{t}") for t in range(TBLK)]
            aoT = [pers.tile([P, L], BF16, tag=f"aoT{m}", name=f"aoT{m}") for m in range(GW // P)]
            ones64 = pers.tile([1, HD], BF16, tag="ones64")
            nc.any.memset(ones64[:], 1.0)
            warm = pers.tile([1, 2], BF16, tag="warm")
            nc.scalar.activation(warm[:], ones64[:, 0:2], EXP)  # preload exp table

            # Input DMA order is the head-phase critical path: the first
            # scores need wq + x cols 0:1024 (q chunks 0,1) + wk.  Weights
            # go on the GpSimd queue so their issue cost doesn't serialize
            # behind the x loads on Sync.
            for k in range(KT):
                nc.gpsimd.dma_start(wqT[k][:], wqT_d[k * P:(k + 1) * P, :])
            for k in range(KT):
                nc.sync.dma_start(xT[k][:, 0:1024], xT_d[k * P:(k + 1) * P, 0:1024])
            for k in range(KT):
                nc.gpsimd.dma_start(wkT[k][:], wkT_d[k * P:(k + 1) * P, :])
            for k in range(KT):
                nc.sync.dma_start(xT[k][:, 1024:L], xT_d[k * P:(k + 1) * P, 1024:L])
            for k in range(KT):
                nc.gpsimd.dma_start(wvT[k][:], wvT_d[k * P:(k + 1) * P, :])
            for i in range(GW // P):
                nc.gpsimd.dma_start(woT[i][:], woT_d[i * P:(i + 1) * P, :])

            # ---- helper emitters ----
            def emit_q_chain(m, tck):
                """q projection for head pair m, chunk tck -> staged into qd."""
                ps = accp.tile([P, 512], F32, tag="fill")
                for k in range(KT):
                    nc.tensor.matmul(
                        ps[:],
                        lhsT=wqT[k][:, m * P:(m + 1) * P],
                        rhs=xT[k][:, tck * 512:(tck + 1) * 512],
                        start=(k == 0),
                        stop=(k == KT - 1),
                    )
                s = qksc.tile([P, 512], BF16, tag="qk")
                nc.vector.tensor_copy(s[:], ps[:])
                po = (tck % 2) * HD
                co = (tck // 2) * 512
                nc.gpsimd.dma_start(qd[2 * m][po:po + HD, co:co + 512], s[0:HD, :])
                nc.gpsimd.dma_start(qd[2 * m + 1][po:po + HD, co:co + 512], s[HD:P, :])

            def emit_k_chain(m, tck):
                """k projection for head pair m, chunk tck -> kd both halves."""
                ps = accp.tile([P, 512], F32, tag="fill")
                for k in range(KT):
                    nc.tensor.matmul(
                        ps[:],
                        lhsT=wkT[k][:, m * P:(m + 1) * P],
                        rhs=xT[k][:, tck * 512:(tck + 1) * 512],
                        start=(k == 0),
                        stop=(k == KT - 1),
                    )
                s = qksc.tile([P, 512], BF16, tag="qk")
                nc.vector.tensor_copy(s[:], ps[:])
                co = tck * 512
                nc.gpsimd.dma_start(kd[2 * m][0:HD, co:co + 512], s[0:HD, :])
                nc.gpsimd.dma_start(kd[2 * m][HD:P, co:co + 512], s[0:HD, :])
                nc.gpsimd.dma_start(kd[2 * m + 1][0:HD, co:co + 512], s[HD:P, :])
                nc.gpsimd.dma_start(kd[2 * m + 1][HD:P, co:co + 512], s[HD:P, :])

            def emit_v_chain(t):
                """vext[t][:, h*65:h*65+64] = (x @ Wv^T)[t-block] per head; col 64 = 1."""
                ps = accp.tile([P, 512], F32, tag="fill")
                for k in range(KT):
                    nc.tensor.matmul(
                        ps[:, :GW],
                        lhsT=xT[k][:, t * P:(t + 1) * P],
                        rhs=wvT[k][:],
                        start=(k == 0),
                        stop=(k == KT - 1),
                    )
                vv = vext[t][:].rearrange("p (h e) -> p h e", h=NH)
                pv = ps[:, :GW].rearrange("p (h e) -> p h e", h=NH)
                nc.vector.tensor_copy(vv[:, :, 0:HD], pv)
                nc.any.memset(vv[:, :, HD:HD + 1], 1.0)

            def emit_scores_exp(h, k):
                """P'[h][k] = exp(SCALE * k-block @ q^T)  -- [128 keys, 2048 q] bf16.

                Row-tiled pairs: the two q-chunks of each half run
                CONCURRENTLY in PE row groups {0,1} / {2,3}."""
                pp = pexp.tile([P, L], BF16, tag="pp")
                for half in range(2):
                    ps = sps.tile([P, 1024], F32, tag="sc", name=f"sc{h}_{k}_{half}")
                    for q in range(2):
                        po = q * HD
                        nc.tensor.matmul(
                            ps[:, q * 512:(q + 1) * 512],
                            lhsT=kd[h][po:po + HD, k * P:(k + 1) * P],
                            rhs=qd[h][po:po + HD, half * 512:(half + 1) * 512],
                            start=True,
                            stop=True,
                        )
                    nc.scalar.activation(
                        pp[:, half * 1024:(half + 1) * 1024], ps[:], EXP, scale=SCALE
                    )
                return pp

            def emit_pv_seg(h, q, pptiles, ov, k0, k1):
                """PV segment: accumulate key-tiles [k0, k1) for (head, chunk)."""
                if ov is None:
                    ov = accp.tile([HD + 1, 512], F32, tag="pv",
                                   name=f"ov{h}_{q}")
                for k in range(k0, k1):
                    nc.tensor.matmul(
                        ov[:],
                        lhsT=vext[k][:, h * (HD + 1):(h + 1) * (HD + 1)],
                        rhs=pptiles[k][:, q * 512:(q + 1) * 512],
                        start=(k == 0),
                        stop=(k == TBLK - 1),
                    )
                return ov

            def emit_oe(ov, act=False):
                oe = oep.tile([HD + 1, 512], BF16, tag="oe")
                if act:
                    nc.scalar.copy(oe[0:HD, :], ov[0:HD, :])
                else:
                    nc.vector.tensor_copy(oe[0:HD, :], ov[0:HD, :])
                return oe

            def emit_norm(h, q, ov, oe):
                """aoT[h-rows, q-chunk] = oe[d, q] * (1/sums)[q] (broadcast over d)."""
                m, off = h // 2, (h % 2) * HD
                srow = srp.tile([1, 512], F32, tag="s")
                nc.vector.tensor_copy(srow[:], ov[HD:HD + 1, :])
                rr = rcpp.tile([1, 512], F32, tag="r")
                nc.vector.reciprocal_approx_fast(rr[:], srow[:])
                rrb = rcpp.tile([1, 512], BF16, tag="rb")
                nc.vector.tensor_copy(rrb[:], rr[:])
                br = accp.tile([HD, 512], F32, tag="fill", name=f"br{h}_{q}")
                nc.tensor.matmul(br[:], lhsT=ones64[:], rhs=rrb[:], start=True, stop=True)
                nc.vector.tensor_mul(
                    aoT[m][off:off + HD, q * 512:(q + 1) * 512],
                    oe[0:HD, :],
                    br[:],
                )

            def emit_pvnorm(h, q, ovs, act=False):
                oe = emit_oe(ovs[q], act=act)
                emit_norm(h, q, ovs[q], oe)

            def emit_oproj(t, evict_act=False, split_dma=False):
                """out[t-block] = ao @ W_o[:, gslice]^T  (partial; host sums groups)."""
                ob = osbp.tile([P, D], BF16, tag="ob")
                for oc in range(2):
                    ps = accp.tile([P, 512], F32, tag="fill")
                    for i in range(GW // P):
                        nc.tensor.matmul(
                            ps[:],
                            lhsT=aoT[i][:, t * P:(t + 1) * P],
                            rhs=woT[i][:, oc * 512:(oc + 1) * 512],
                            start=(i == 0),
                            stop=(i == GW // P - 1),
                        )
                    if evict_act and oc == 0:
                        nc.scalar.copy(ob[:, oc * 512:(oc + 1) * 512], ps[:])
                    else:
                        nc.vector.tensor_copy(ob[:, oc * 512:(oc + 1) * 512], ps[:])
                    if split_dma:
                        for g in range(2):
                            nc.sync.dma_start(
                                out_d[t * P + g * 64:t * P + (g + 1) * 64,
                                      oc * 512:(oc + 1) * 512],
                                ob[g * 64:(g + 1) * 64, oc * 512:(oc + 1) * 512],
                            )
                    else:
                        nc.sync.dma_start(
                            out_d[t * P:(t + 1) * P, oc * 512:(oc + 1) * 512],
                            ob[:, oc * 512:(oc + 1) * 512],
                        )

            # ---- emission schedule ----
            # q/k chains needed by the first scores: all of q(m=0) and the
            # first column-chunk of k(m=0).
            emit_q_chain(0, 0)
            emit_q_chain(0, 1)
            emit_k_chain(0, 0)

            # Remaining projection work spread across the head iterations as
            # PE fillers.  All v chains must land in head 0: head 1's PV
            # segments read vext from step 0.  q(0,2)/q(0,3) come first --
            # the first score pair's second half reads them.
            fillers = {0: [], 1: [], 2: [], 3: []}
            for tcx in (2, 3):
                fillers[0].append(lambda tcx=tcx: emit_q_chain(0, tcx))
            for tcx in range(1, QC):
                fillers[0].append(lambda tcx=tcx: emit_k_chain(0, tcx))
            for t in range(TBLK):
                fillers[0].append(lambda t=t: emit_v_chain(t))
            for tcx in range(QC):
                fillers[1].append(lambda tcx=tcx: emit_q_chain(1, tcx))
            for tcx in range(QC):
                fillers[1].append(lambda tcx=tcx: emit_k_chain(1, tcx))

            # Per head iteration: 16 k-steps.  Each step emits (PE order)
            # the PV segment of the previous head, then the score pair +
            # exps -- so P' slots freed by the segment are available to the
            # step's exp, and the PE never bursts >~2us without producing a
            # score tile for ACT.  PV chunk schedule: chunks 0,1 alternate
            # 4-MM segments over steps 0-7; chunks 2,3 over steps 8-15.
            # Chunk chains complete at steps 6,7,14,15; norms follow one
            # step later (chunk 3's norm lands after the loop).
            pp_prev = None
            pp_cur = []
            for h in range(NH):
                hp = h - 1
                ovs = [None] * QC
                fi = 0
                for k in range(TBLK):
                    if h > 0:
                        q = (k // 8) * 2 + (k % 2)
                        seg = (k % 8) // 2
                        ovs[q] = emit_pv_seg(hp, q, pp_prev, ovs[q],
                                             seg * 4, seg * 4 + 4)
                    pp_cur.append(emit_scores_exp(h, k))
                    if h > 0:
                        if k == 7:
                            emit_pvnorm(hp, 0, ovs)
                        elif k == 8:
                            emit_pvnorm(hp, 1, ovs)
                        elif k == 15:
                            emit_pvnorm(hp, 2, ovs)
                    # filler pacing: stay on schedule across the 16 steps
                    nf = len(fillers[h])
                    if nf:
                        tgt = ((k + 1) * nf + TBLK - 1) // TBLK
                        while fi < min(tgt, nf):
                            fillers[h][fi]()
                            fi += 1
                if h > 0:
                    emit_pvnorm(hp, 3, ovs)
                for f in fillers[h][fi:]:
                    f()
                pp_prev = pp_cur
                pp_cur = []

            # ---- tail: PV/norm for head 3 + output projection ----
            h3 = NH - 1
            ovs = [None] * QC
            for seg in range(4):
                ovs[0] = emit_pv_seg(h3, 0, pp_prev, ovs[0], seg * 4, seg * 4 + 4)
                ovs[1] = emit_pv_seg(h3, 1, pp_prev, ovs[1], seg * 4, seg * 4 + 4)
            emit_pvnorm(h3, 0, ovs)          # DVE evict: ACT still on last exps
            emit_pvnorm(h3, 1, ovs)
            for seg in range(4):
                ovs[2] = emit_pv_seg(h3, 2, pp_prev, ovs[2], seg * 4, seg * 4 + 4)
                ovs[3] = emit_pv_seg(h3, 3, pp_prev, ovs[3], seg * 4, seg * 4 + 4)
                emit_oproj(seg, evict_act=False)
            emit_pvnorm(h3, 2, ovs, act=False)
            for t in range(4, 6):
                emit_oproj(t, evict_act=False)
            for t in range(6, 8):
                emit_oproj(t, evict_act=True)
            emit_pvnorm(h3, 3, ovs, act=True)
            for t in range(8, 12):
                emit_oproj(t, evict_act=True)
            for t in range(12, TBLK):
                emit_oproj(t, evict_act=True, split_dma=True)
    nc.compile()
    return nc


_NC = None


def _get_nc():
    global _NC
    if _NC is None:
        _NC = _build()
    return _NC


def _shard(inputs):
    x = np.asarray(inputs["x"], dtype=np.float32)
    W_q = np.asarray(inputs["W_q"], dtype=np.float32)
    W_k = np.asarray(inputs["W_k"], dtype=np.float32)
    W_v = np.asarray(inputs["W_v"], dtype=np.float32)
    W_o = np.asarray(inputs["W_o"], dtype=np.float32)
    bf = ml_dtypes.bfloat16
    in_maps = []
    for core in range(8):
        b, g = core // 4, core % 4
        sl = slice(g * GW, (g + 1) * GW)
        in_maps.append({
            "xT": np.ascontiguousarray(x[b].T).astype(bf),
            "wqT": np.ascontiguousarray(W_q[sl, :].T).astype(bf),
            "wkT": np.ascontiguousarray(W_k[sl, :].T).astype(bf),
            "wvT": np.ascontiguousarray(W_v[sl, :].T).astype(bf),
            "woT": np.ascontiguousarray(W_o[:, sl].T).astype(bf),
        })
    return in_maps


def _run(inputs, trace=False):
    nc = _get_nc()
    in_maps = _shard(inputs)
    res = run_bass_kernel_spmd(nc, in_maps, core_ids=list(range(8)), trace=trace)
    out = np.zeros((B, L, D), dtype=np.float32)
    for core in range(8):
        out[core // 4] += res.results[core]["out"].astype(np.float32)
    return out, res


def kernel(**inputs) -> np.ndarray:
    out, _ = _run(inputs, trace=False)
    return out
